# revision 43
# baseline (speedup 1.0000x reference)
"""AttentionFlowLayer (BiDAF-style) Trainium2 kernel.

Full inputs in, full output out. Data-parallel over batch B=32 across 8
NeuronCores (4 batches per core, no cross-core communication).

Math (per batch b):
    S[i,j]  = main[i,j] + hw[i] + uw[j] + b,  main = (h * w_hu) @ u^T
    a[i,j]  = softmax_j(where(u_mask, S, NEG))      -> hw[i], b cancel
    b_t[i,j]= softmax_i(where(h_mask, S, NEG))      -> uw[j], b cancel
    U~ = a @ u ; H~ = b_t @ (a^T @ h)               (avoids [Lh,Lh] interm.)
    out = [h, U~, h*U~, h*H~]

Design notes:
  * All-bf16 compute (tolerance 2e-2; measured pipeline rel err ~8e-4).
  * S is computed TRANSPOSED: S^T = (u*w_hu) @ h^T; E^T = exp(S^T + uwm)
    with uwm as a per-partition ACT bias. E^T tiles serve directly as
    matmul weights (lhsT) for the U~ and H~-apply matmuls.
  * Unnormalized-softmax algebra (no max pass; exponents are O(10)):
        s[i] = sum_j E^T[j,i] (per-tile ones matmuls), r = 1/s
        U~ = (E @ u) * r
        a_nat = PE-transpose(E^T) * r  (one fused DVE scaled exit)
        G = a_nat^T @ h ; Z = a_nat^T @ (eb*s)  (= sum_i E[i,j] eb[i])
        G'' = G/(Z+tiny) ;  h*H~ = (h*eb) * (E @ G'')
  * 3-stage software pipeline (S1: S^T/exp/s; S2: transpose, U~, h*U~,
    G|Z, G''; S3: H~-apply + store), emitted interleaved across batches
    so no engine queue head-of-line blocks on a late dependency, with
    the heavy G-chain pulled out of the final stage so the pipeline
    tail is short; PSUM fits exactly in 8 banks. Input prefetches are
    spread over the sync/scalar/gpsimd queues, and stores drain in
    pieces as each chunk of h*H~ completes.
  * All HBM traffic is bf16: inputs are host-packed into one [128, 4616]
    bf16 + one [128, 9] f32 buffer per batch (2 DMA loads, hoisted ahead
    of the loop so no input transfer ever queues behind an output store
    on the in-order sync queue), device output is bf16 [1024, 768]
    (cols H:4H; host upcasts), and out[:, 0:H] = h is filled by the
    host in f32 (exact, zero device traffic). ~10.6 MB HBM per core.
"""
import sys

if "/opt/trn_rl_repo" not in sys.path:
    sys.path.insert(0, "/opt/trn_rl_repo")

import numpy as np
from contextlib import ExitStack

import concourse.bass as bass
import concourse.bacc as bacc
import concourse.tile as tile
from concourse import mybir
from concourse.bass_utils import run_bass_kernel_spmd
from concourse.masks import make_identity

B, LH, LU, H = 32, 1024, 128, 256
NCORES = 8
BP = B // NCORES          # batches per core
NT = LH // 128            # 8 i-tiles of 128 rows
NP = NT // 2              # i-tile pairs
NEG = -1e30

F32 = mybir.dt.float32
BF16 = mybir.dt.bfloat16
ts = bass.ts
EXP = mybir.ActivationFunctionType.Exp

# packed bf16 input layout (per partition, in elements). Part A (what
# the S^T matmul needs first) loads ahead of part B.
PK_HT = 0           # hT  [2, 1024] (c = k*128 + p)
PK_UTW = 2048       # uTw [2, 128]  (c = k*128 + p)
PK_A = 2304         # end of part A
PK_U = 2304         # u   [256]     (j = p)
PK_EB = 2560        # eb  [8] bf16  (i = t*128 + p)
PK_H = 2568         # h   [8, 256] (i = t*128 + p)
PK16_N = 4616
# packed f32 layout
PK_EB32 = 0         # eb  [8] f32
PK_UWM = 8          # uwm [1] (j = p)
PK32_N = 9


def _body(tc):
    nc = tc.nc
    pk16_ext = nc.declare_dram_parameter("pk16", [BP, 128, PK16_N], BF16, isOutput=False)
    pk32_ext = nc.declare_dram_parameter("pk32", [BP, 128, PK32_N], F32, isOutput=False)
    out_ext = nc.declare_dram_parameter("out", [BP, LH, 3 * H], BF16, isOutput=True)

    with ExitStack() as ctx:
        const = ctx.enter_context(tc.tile_pool(name="const", bufs=1))
        p_in16 = ctx.enter_context(tc.tile_pool(name="p_in16", bufs=BP))
        p_in32 = ctx.enter_context(tc.tile_pool(name="p_in32", bufs=BP))
        p_ET = ctx.enter_context(tc.tile_pool(name="p_ET", bufs=3))
        p_Enr = ctx.enter_context(tc.tile_pool(name="p_Enr", bufs=2))
        p_o = ctx.enter_context(tc.tile_pool(name="p_o", bufs=3))
        p_small = ctx.enter_context(tc.tile_pool(name="p_small", bufs=8))
        ps_ST = ctx.enter_context(tc.tile_pool(name="ps_ST", bufs=1, space="PSUM"))
        ps_s = ctx.enter_context(tc.tile_pool(name="ps_s", bufs=2, space="PSUM"))
        ps_tp = ctx.enter_context(tc.tile_pool(name="ps_tp", bufs=1, space="PSUM"))
        ps_mm = ctx.enter_context(tc.tile_pool(name="ps_mm", bufs=2, space="PSUM"))
        ps_G = ctx.enter_context(tc.tile_pool(name="ps_G", bufs=1, space="PSUM"))

        ones_bf = const.tile([128, 1], BF16)
        nc.vector.memset(ones_bf, 1.0)
        ident_bf = const.tile([128, 128], BF16)
        make_identity(nc, ident_bf)

        # prefetch all per-batch inputs up front so input DMAs are never
        # queued behind an output DMA on the in-order sync queue
        pk16s, pk32s = [], []
        for bb in range(BP):
            pk16 = p_in16.tile([128, PK16_N], BF16)
            pk32 = p_in32.tile([128, PK32_N], F32)
            pk16s.append(pk16)
            pk32s.append(pk32)
        # batch 0's S^T operands first, then the rest; pk32/later pk16
        # issues go out on the scalar/gpsimd queues in parallel
        nc.sync.dma_start(out=pk16s[0][:, 0:PK_A], in_=pk16_ext[0, :, 0:PK_A])
        nc.sync.dma_start(out=pk16s[0][:, PK_A:], in_=pk16_ext[0, :, PK_A:])
        nc.sync.dma_start(out=pk16s[1], in_=pk16_ext[1])
        nc.scalar.dma_start(out=pk16s[2], in_=pk16_ext[2])
        nc.scalar.dma_start(out=pk16s[3], in_=pk16_ext[3])
        for bb in range(BP):
            nc.gpsimd.dma_start(out=pk32s[bb], in_=pk32_ext[bb])

        def views(bb):
            pk16, pk32 = pk16s[bb], pk32s[bb]
            return {
                "h": pk16[:, PK_H : PK_H + NT * H].rearrange("p (t c) -> p t c", t=NT),
                "hT": pk16[:, PK_HT : PK_HT + 2048].rearrange("p (k i) -> p k i", k=2),
                "uTw": pk16[:, PK_UTW : PK_UTW + 256].rearrange("p (k j) -> p k j", k=2),
                "u": pk16[:, PK_U : PK_U + H],
                "ebbf": pk16[:, PK_EB : PK_EB + NT],
                "eb32": pk32[:, PK_EB32 : PK_EB32 + NT],
                "uwm": pk32[:, PK_UWM : PK_UWM + 1],
            }

        st1, st2 = {}, {}

        def stage1(bb):
            v = views(bb)
            # S^T = (u*w_hu) @ h^T : [128 j, 1024 i] f32 PSUM
            st_ps = ps_ST.tile([128, LH], F32)
            for hh in range(2):
                for k in range(2):
                    nc.tensor.matmul(
                        st_ps[:, ts(hh, 512)],
                        v["uTw"][:, k, :],
                        v["hT"][:, k, ts(hh, 512)],
                        start=(k == 0),
                        stop=(k == 1),
                    )
            # E^T = exp(S^T + uwm[j]) -> bf16 SBUF (halves: the s
            # matmuls of the first half start under the second half)
            ET = p_ET.tile([128, LH], BF16)
            for hh in range(2):
                nc.scalar.activation(
                    ET[:, ts(hh, 512)], st_ps[:, ts(hh, 512)], EXP, bias=v["uwm"]
                )
            # s[i] = col sums of E^T (per i-tile ones matmuls)
            s_ps = ps_s.tile([128, NT], F32)
            for t in range(NT):
                nc.tensor.matmul(s_ps[:, t : t + 1], ET[:, ts(t, 128)], ones_bf)
            st1[bb] = (ET, s_ps)

        def stage2(bb):
            v = views(bb)
            ET, s_ps = st1.pop(bb)
            r_sb = p_small.tile([128, NT], F32)
            nc.vector.reciprocal(r_sb, s_ps)
            # ebs = eb * s (bf16): the Z matmul rhs against a_nat weights
            ebs = p_small.tile([128, NT], BF16)
            nc.vector.tensor_mul(ebs, s_ps, v["eb32"])

            # a_nat = transpose(E^T) * r via PE transposes + one fused
            # DVE scaled exit (broadcast r over the row dim)
            tp_ps = ps_tp.tile([128, NT, 128], BF16)
            for t in range(NT):
                nc.tensor.transpose(tp_ps[:, t, :], ET[:, ts(t, 128)], ident_bf)
            Enr = p_Enr.tile([128, NT, LU], BF16)
            nc.vector.tensor_mul(Enr, tp_ps, r_sb.broadcast_to((128, NT, LU)))

            o_sb = p_o.tile([128, NT, 3 * H], BF16)
            # heb = h * eb, staged straight into the h*H~ block; split
            # 5/3 between gpsimd and DVE to balance engine walls
            nc.gpsimd.tensor_mul(
                o_sb[:, 0:5, 2 * H : 3 * H],
                v["h"][:, 0:5, :],
                v["eb32"][:, 0:5].broadcast_to((128, 5, H)),
            )
            nc.vector.tensor_mul(
                o_sb[:, 5:NT, 2 * H : 3 * H],
                v["h"][:, 5:NT, :],
                v["eb32"][:, 5:NT].broadcast_to((128, NT - 5, H)),
            )

            # U~ = (E @ u) * r
            for p in range(NP):
                eu = ps_mm.tile([128, 2, H], F32, tag="mm")
                for q in range(2):
                    t = 2 * p + q
                    nc.tensor.matmul(eu[:, q, :], ET[:, ts(t, 128)], v["u"])
                for q in range(2):
                    t = 2 * p + q
                    nc.scalar.mul(o_sb[:, t, 0:H], eu[:, q, :], r_sb[:, t : t + 1])

            # h*U~, split between gpsimd and DVE to balance engine walls
            nc.gpsimd.tensor_mul(
                o_sb[:, 0:4, H : 2 * H], v["h"][:, 0:4, :], o_sb[:, 0:4, 0:H]
            )
            nc.vector.tensor_mul(
                o_sb[:, 4:NT, H : 2 * H], v["h"][:, 4:NT, :], o_sb[:, 4:NT, 0:H]
            )
            if bb == BP - 1:
                # final batch: U~ and h*U~ columns are complete ~3us
                # before h*H~ -- drain them to HBM early
                nc.sync.dma_start(
                    out=out_ext[bb, :, 0 : 2 * H].rearrange(
                        "(t p) c -> p t c", p=128
                    ),
                    in_=o_sb[:, :, 0 : 2 * H],
                )

            # G = a_nat^T @ h ; Z = a_nat^T @ (eb*s) into col H. Runs here
            # (not stage3) so the pipeline's last units are small.
            g_ps = ps_G.tile([128, H + 1], F32)
            for t in range(NT):
                nc.tensor.matmul(
                    g_ps[:, 0:H],
                    Enr[:, t, :],
                    v["h"][:, t, :],
                    start=(t == 0),
                    stop=(t == NT - 1),
                )
            for t in range(NT):
                nc.tensor.matmul(
                    g_ps[:, H : H + 1],
                    Enr[:, t, :],
                    ebs[:, t : t + 1],
                    start=(t == 0),
                    stop=(t == NT - 1),
                )
            rz = p_small.tile([128, 1], F32)
            nc.vector.tensor_scalar_add(rz, g_ps[:, H : H + 1], 1e-30)
            nc.vector.reciprocal(rz, rz)
            gpp = p_small.tile([128, H], BF16)
            nc.vector.tensor_scalar_mul(gpp, g_ps[:, 0:H], rz)
            st2[bb] = (ET, gpp, o_sb)

        def stage3(bb):
            ET, gpp, o_sb = st2.pop(bb)

            # h*H~ = heb * (E @ G'') (in place over the heb block); the
            # store of each finished piece is issued immediately so the
            # HBM write drain overlaps the remaining compute (halves;
            # quarters for the final batch)
            last = bb == BP - 1
            ppp = 1 if last else 2  # pairs per stored piece
            c0 = 2 * H if last else 0  # U~|h*U~ already stored for last
            for p in range(NP):
                ag = ps_mm.tile([128, 2, H], F32, tag="mm")
                for q in range(2):
                    nc.tensor.matmul(ag[:, q, :], ET[:, ts(2 * p + q, 128)], gpp)
                nc.vector.tensor_mul(
                    o_sb[:, 2 * p : 2 * p + 2, 2 * H : 3 * H],
                    o_sb[:, 2 * p : 2 * p + 2, 2 * H : 3 * H],
                    ag,
                )
                if (p + 1) % ppp == 0:
                    qq = p // ppp
                    step = 2 * ppp
                    nc.sync.dma_start(
                        out=out_ext[bb, ts(qq, step * 128), c0 : 3 * H].rearrange(
                            "(t p) c -> p t c", p=128
                        ),
                        in_=o_sb[:, qq * step : (qq + 1) * step, c0 : 3 * H],
                    )

        # 3-deep software pipeline across batches
        sched = [
            (stage1, 0), (stage1, 1), (stage2, 0), (stage1, 2), (stage2, 1),
            (stage3, 0), (stage1, 3), (stage2, 2), (stage3, 1), (stage2, 3),
            (stage3, 2), (stage3, 3),
        ]
        for fn, bb in sched:
            fn(bb)


_NC_CACHE = None


def _build_nc():
    global _NC_CACHE
    if _NC_CACHE is None:
        nc = bacc.Bacc("TRN2", target_bir_lowering=False, enable_partition_id=False)
        with tile.TileContext(nc) as tc:
            _body(tc)
        nc.finalize()
        _NC_CACHE = nc
    return _NC_CACHE


def _make_in_maps(h, u, h_mask, u_mask, w, b):
    import ml_dtypes

    bf16 = ml_dtypes.bfloat16
    h = np.ascontiguousarray(h, dtype=np.float32)
    u = np.ascontiguousarray(u, dtype=np.float32)
    w = np.asarray(w, dtype=np.float32)
    w_h, w_u, w_hu = w[:H], w[H : 2 * H], w[2 * H :]
    eb = np.where(h_mask, np.exp(h @ w_h), np.float32(0.0)).astype(np.float32)
    uwm = (u @ w_u + np.where(u_mask, np.float32(0.0), np.float32(NEG))).astype(
        np.float32
    )

    pk16 = np.zeros((B, 128, PK16_N), bf16)
    hT = h.transpose(0, 2, 1)
    pk16[:, :, PK_HT : PK_HT + 2048] = (
        hT.reshape(B, 2, 128, LH).transpose(0, 2, 1, 3).reshape(B, 128, 2048)
    )
    uTw = (u * w_hu).transpose(0, 2, 1)
    pk16[:, :, PK_UTW : PK_UTW + 256] = (
        uTw.reshape(B, 2, 128, LU).transpose(0, 2, 1, 3).reshape(B, 128, 256)
    )
    pk16[:, :, PK_U : PK_U + H] = u
    pk16[:, :, PK_EB : PK_EB + NT] = eb.reshape(B, NT, 128).transpose(0, 2, 1)
    pk16[:, :, PK_H : PK_H + NT * H] = (
        h.reshape(B, NT, 128, H).transpose(0, 2, 1, 3).reshape(B, 128, NT * H)
    )
    pk32 = np.zeros((B, 128, PK32_N), np.float32)
    pk32[:, :, PK_EB32 : PK_EB32 + NT] = eb.reshape(B, NT, 128).transpose(0, 2, 1)
    pk32[:, :, PK_UWM] = uwm

    in_maps = []
    for i in range(NCORES):
        s = slice(i * BP, (i + 1) * BP)
        in_maps.append({"pk16": pk16[s], "pk32": pk32[s]})
    return in_maps


def _assemble(h, results):
    out = np.empty((B, LH, 4 * H), np.float32)
    out[:, :, 0:H] = h
    dev = np.concatenate([results[i]["out"] for i in range(NCORES)], axis=0)
    out[:, :, H:] = dev.astype(np.float32)
    return out


def kernel(h, u, h_mask, u_mask, w, b):
    nc = _build_nc()
    in_maps = _make_in_maps(h, u, h_mask, u_mask, w, b)
    res = run_bass_kernel_spmd(nc, in_maps, core_ids=list(range(NCORES)))
    return _assemble(np.asarray(h, np.float32), res.results)
